# revision 15
# baseline (speedup 1.0000x reference)
"""Multi-head attention with KV cache on 8 Trainium2 NeuronCores.

Sharding (per the hint): data-parallel over the batch (2 groups of 4
cores), tensor-parallel over the 16 heads (4 heads per core).  Attention
is fully local per head; the output projection is column-split 4-ways
within each batch group after an AllGather of the per-head contexts.

Layout strategy: everything flows through the PE in "transposed"
orientation so no on-device transposes are needed:
  qkT[e, s] = (w_qk chunks).T @ xT chunks
  scoresT[t, s] = kT-chunk.T @ qT          (softmax free-dim = s)
  ctxT'[d+1, s] = [v | 1].T-chunks @ expT  (row 64 = softmax denominator)
  outT[oc, s] = wproj-chunk.T @ mergedT
v is additionally produced in natural [s, d] orientation (separate
tiling of the same projection) because present-v and the ctx matmul's
stationary operand both want it that way.

Head PAIRS are processed together: scores for the two heads run as
concurrent 64x128 row-tiles of the PE array (tile_position), and one
wide ACTIVATE handles both heads' exp.  All matmuls run in float32r
(TF32-like, full PE rate, ~1.5e-4 scaled error).  The causal mask is
applied structurally: fully-masked t-chunks are skipped and the
diagonal 128x128 block is one multiplicative tri pattern.  The host
verifies the mask really has this structure and falls back to a general
multiplicative-mask path (or numpy for degenerate masks) otherwise.
"""
import os
import sys

sys.path.insert(0, "/opt/trn_rl_repo")

import numpy as np

B, S, D, H, P = 2, 2048, 1024, 16, 2048
T = P + S
DEPTH = D // H          # 64
N_CORES = 8
GROUPS = [[0, 1, 2, 3], [4, 5, 6, 7]]
NH = H // 4             # 4 local heads per core
EQK = 2 * NH * DEPTH    # 512 (q block 256 | k block 256)
EV = NH * DEPTH         # 256
OC = D // 4             # 256 output columns per core
SN = 512                # attention s-tile width
NSN = S // SN           # 4
NTCH = T // 128         # 32 t-chunks
KCH = D // 128          # 8 contraction chunks

_prog_cache = {}
LAST_RESULT = None


def _build_plan(mask2d, tri_expect):
    """Per (s-tile, t-chunk) mask classification.

    plan[sn] = list of (t0, state); state is ('full',), ('tri', off) or
    ('dram',).  Fully-masked chunks are dropped.  mode is 'causal' when
    no 'dram' states exist.
    """
    plan = []
    mode = "causal"
    for sn in range(NSN):
        s0 = sn * SN
        chunks = []
        for t0 in range(0, T, 128):
            sub = mask2d[s0:s0 + SN, t0:t0 + 128]
            if not sub.any():
                chunks.append((t0, ("full",)))
                continue
            if sub.all():
                continue
            st = None
            subT = sub.T  # [t 128, s SN]
            for off in range(0, SN, 128):
                cand = np.ones((128, SN), dtype=np.float32)
                cand[:, :off] = 0.0
                cand[:, off:off + 128] = tri_expect
                if np.array_equal(1.0 - subT, cand):
                    st = ("tri", off)
                    break
            if st is None:
                st = ("dram",)
                mode = "general"
            chunks.append((t0, st))
        plan.append(chunks)
    return plan, mode


def _plan_key(plan, mode, has_bias):
    return (mode, has_bias,
            tuple((sn_i, t0, st)
                  for sn_i, chunks in enumerate(plan)
                  for t0, st in chunks))


def _build_program(plan, mode, has_bias):
    import concourse.bacc as bacc
    import concourse.mybir as mybir
    from concourse import tile as tile_mod
    from concourse.tile import add_dep_helper

    f32 = mybir.dt.float32
    f32r = mybir.dt.float32r
    AF = mybir.ActivationFunctionType

    nc = bacc.Bacc(None)

    # ---- per-core external tensors -------------------------------------
    xbT_d = nc.dram_tensor("xbT", [D, S], f32, kind="ExternalInput")
    wqk_d = nc.dram_tensor("wqk", [D, EQK], f32, kind="ExternalInput")
    wv_d = nc.dram_tensor("wv", [D, EV], f32, kind="ExternalInput")
    pastKT_d = nc.dram_tensor("pastKT", [NH, DEPTH, P], f32, kind="ExternalInput")
    pastV_d = nc.dram_tensor("pastV", [NH, P, DEPTH], f32, kind="ExternalInput")
    tri_d = nc.dram_tensor("tri", [128, 128], f32, kind="ExternalInput")
    wproj_d = nc.dram_tensor("wproj", [D, OC], f32, kind="ExternalInput")
    if has_bias:
        bqk_d = nc.dram_tensor("bqk", [EQK], f32, kind="ExternalInput")
        bv_d = nc.dram_tensor("bv", [EV], f32, kind="ExternalInput")
        bproj_d = nc.dram_tensor("bproj", [OC], f32, kind="ExternalInput")
    if mode == "general":
        multT_d = nc.dram_tensor("multT", [T, S], f32, kind="ExternalInput")

    pkT_d = nc.dram_tensor("pkT", [NH, DEPTH, S], f32, kind="ExternalOutput")
    pv_d = nc.dram_tensor("pv", [NH, S, DEPTH], f32, kind="ExternalOutput")
    outT_d = nc.dram_tensor("outT", [OC, S], f32, kind="ExternalOutput")

    with tile_mod.TileContext(nc) as tc:
        with (
            tc.tile_pool(name="sb", bufs=1) as sb,
            tc.tile_pool(name="ps", bufs=1, space="PSUM") as ps,
            tc.tile_pool(name="dram", bufs=1, space="DRAM") as dram,
        ):
            cc_in = [dram.tile([EV, 1536], f32, name="cc_in0"),
                     dram.tile([EV, 512], f32, name="cc_in1")]
            cc_out = [dram.tile([D, 1536], f32, name="cc_out0"),
                      dram.tile([D, 512], f32, name="cc_out1")]

            def stg(shape, name):
                return sb.tile(shape, f32, name=name, tag="stg",
                               bufs=4, padded_shape=[128, 2048])

            # ---- small constants ----------------------------------------
            tri_s = stg([128, 128], "tri_s")
            nc.sync.dma_start(out=tri_s[:], in_=tri_d[:])
            tri_r = sb.tile([128, 128], f32r, name="tri_r")
            nc.vector.tensor_copy(tri_r[:], tri_s[:])
            onec = sb.tile([128, 1], f32, name="onec")
            nc.gpsimd.memset(onec[:], 1.0)

            if has_bias:
                bqk_s = sb.tile([128, EQK // 128], f32, name="bqk_s")
                nc.sync.dma_start(out=bqk_s[:],
                                  in_=bqk_d.rearrange("(t p) -> p t", p=128))
                bpr_s = sb.tile([128, OC // 128], f32, name="bpr_s")
                nc.sync.dma_start(out=bpr_s[:],
                                  in_=bproj_d.rearrange("(t p) -> p t", p=128))
                bv_row = sb.tile([1, EV], f32, name="bv_row")
                nc.sync.dma_start(out=bv_row[:], in_=bv_d[None, :])
                bv_b = sb.tile([128, EV], f32, name="bv_b")
                nc.gpsimd.partition_broadcast(bv_b[:], bv_row[:])

            # ---- weights: load + round ----------------------------------
            wqkr = []
            for k in range(KCH):
                st = stg([128, EQK], f"wqkst{k}")
                nc.sync.dma_start(out=st[:], in_=wqk_d[k * 128:(k + 1) * 128, :])
                wr = sb.tile([128, EQK], f32r, name=f"wqkr{k}", tag="wqk", bufs=KCH)
                nc.vector.tensor_copy(wr[:], st[:])
                wqkr.append(wr)
            wvr = []
            for k in range(KCH):
                st = stg([128, EV], f"wvst{k}")
                nc.sync.dma_start(out=st[:], in_=wv_d[k * 128:(k + 1) * 128, :])
                wr = sb.tile([128, EV], f32r, name=f"wvr{k}", tag="wv", bufs=KCH)
                nc.vector.tensor_copy(wr[:], st[:])
                wvr.append(wr)

            # ---- phases 1+2 interleaved by head pair --------------------
            # For each pair: QKV projections for its heads, then its
            # first-half attention (overlaps the other pair's phase 1 /
            # the ACT-bound attention hides the PE/DMA work).
            kbufs, qbufs, vbufs = [None] * 2, [None] * 2, [None] * 2
            last_insts = {}

            def attn(sn, p):
                s0 = sn * SN
                chunks = plan[sn]
                last_ci = len(chunks) - 1
                kbuf, qbuf, vbuf = kbufs[p], qbufs[p], vbufs[p]
                ctxps = [ps.tile([65, SN], f32, name=f"ctx{sn}_{p}_{i}",
                                 tag=("ctxA" if i == 0 else "ctxB"), bufs=2)
                         for i in range(2)]
                for ci, (t0, st_h) in enumerate(chunks):
                    tci = t0 // 128
                    scp = ps.tile([128, 2 * SN], f32, name=f"sc{sn}_{p}_{tci}",
                                  tag="scpT", bufs=2)
                    for i, (r0, tp) in enumerate(((0, (0, 0)), (64, (64, 0)))):
                        nc.tensor.matmul(
                            scp[:, i * SN:(i + 1) * SN],
                            kbuf[r0:r0 + 64, t0:t0 + 128],
                            qbuf[r0:r0 + 64, s0:s0 + SN],
                            start=True, stop=True, tile_position=tp)
                    expt = sb.tile([128, 2 * SN], f32r, name=f"ex{sn}_{p}_{tci}",
                                   tag="expt", bufs=3)
                    nc.scalar.activation(expt[:], scp[:], AF.Exp, scale=0.125)
                    mrr = None
                    for i in range(2):
                        base = i * SN
                        if st_h[0] == "tri":
                            off = st_h[1]
                            if off > 0:
                                nc.vector.tensor_scalar_mul(
                                    expt[:, base:base + off],
                                    expt[:, base:base + off], 0.0)
                            nc.vector.tensor_mul(
                                out=expt[:, base + off:base + off + 128],
                                in0=expt[:, base + off:base + off + 128],
                                in1=tri_r[:])
                        elif st_h[0] == "dram":
                            if mrr is None:
                                mst = stg([128, SN], f"mst{sn}_{p}_{tci}")
                                nc.sync.dma_start(
                                    out=mst[:],
                                    in_=multT_d[t0:t0 + 128, s0:s0 + SN])
                                mrr = sb.tile([128, SN], f32r,
                                              name=f"mrr{sn}_{p}_{tci}",
                                              tag="mrr", bufs=2)
                                nc.vector.tensor_copy(mrr[:], mst[:])
                            nc.vector.tensor_mul(
                                out=expt[:, base:base + SN],
                                in0=expt[:, base:base + SN],
                                in1=mrr[:])
                    for i in range(2):
                        nc.tensor.matmul(
                            ctxps[i][:],
                            vbuf[:, (i * NTCH + tci) * 65:
                                 (i * NTCH + tci) * 65 + 65],
                            expt[:, i * SN:(i + 1) * SN],
                            start=(ci == 0), stop=(ci == last_ci))
                for i in range(2):
                    h = 2 * p + i
                    ctxs = sb.tile([65, SN], f32, name=f"ctxs{sn}_{p}_{i}",
                                   tag="ctxs", bufs=3)
                    nc.vector.tensor_copy(ctxs[:], ctxps[i][:])
                    rec = sb.tile([1, SN], f32, name=f"rec{sn}_{p}_{i}",
                                  tag="rec", bufs=2)
                    nc.vector.reciprocal(out=rec[:], in_=ctxs[64:65, :])
                    recb = sb.tile([64, SN], f32, name=f"recb{sn}_{p}_{i}",
                                   tag="recb", bufs=2)
                    nc.gpsimd.partition_broadcast(recb[:], rec[:])
                    mstg = sb.tile([64, SN], f32, name=f"mstg{sn}_{p}_{i}",
                                   tag="mstg", bufs=3)
                    last_insts['dve'] = nc.vector.tensor_mul(
                        out=mstg[:], in0=ctxs[0:64, :], in1=recb[:])
                    w_i, c_i2 = (0, sn * SN) if sn < 3 else (1, 0)
                    last_insts['dve' + str(w_i)] = last_insts['dve']
                    last_insts['dma' + str(w_i)] = last_insts['dma'] = nc.sync.dma_start(
                        out=cc_in[w_i][h * 64:(h + 1) * 64, c_i2:c_i2 + SN],
                        in_=mstg[:])

            for p in range(2):
                kbufs[p] = sb.tile([128, T], f32r, name=f"kbuf{p}", tag="kbuf", bufs=2)
                qbufs[p] = sb.tile([128, S], f32r, name=f"qbuf{p}", tag="qbuf", bufs=2)
                vbufs[p] = sb.tile([128, 2 * NTCH * 65], f32r, name=f"vbuf{p}",
                                   tag="vbuf", bufs=2)

            # ---- phase 1: QKV projections (both pairs) ------------------
            for sq in range(4):         # s quarters of 512
                xbr = []
                for k in range(KCH):
                    st = stg([128, 512], f"xbst{sq}_{k}")
                    nc.sync.dma_start(
                        out=st[:], in_=xbT_d[k * 128:(k + 1) * 128,
                                             sq * 512:(sq + 1) * 512])
                    xr = sb.tile([128, 512], f32r, name=f"xbr{sq}_{k}",
                                 tag="xb", bufs=KCH + 1)
                    nc.vector.tensor_copy(xr[:], st[:])
                    xbr.append(xr)
                for e in range(4):
                    pq = ps.tile([128, 512], f32, name=f"pqk{sq}_{e}",
                                 tag=("ctxA" if e % 2 == 0 else "ctxB"), bufs=2)
                    for k in range(KCH):
                        nc.tensor.matmul(pq[:], wqkr[k][:, e * 128:(e + 1) * 128],
                                         xbr[k][:], start=(k == 0),
                                         stop=(k == KCH - 1))
                    if e < 2:
                        dst = qbufs[e][:, sq * 512:(sq + 1) * 512]
                    else:
                        dst = kbufs[e - 2][:, P + sq * 512:P + (sq + 1) * 512]
                    if has_bias:
                        nc.vector.tensor_scalar_add(dst, pq[:], bqk_s[:, e:e + 1])
                    else:
                        nc.vector.tensor_copy(dst, pq[:])
                    if e >= 2:
                        j = e - 2
                        nc.sync.dma_start(
                            out=pkT_d[2 * j, :, sq * 512:(sq + 1) * 512],
                            in_=kbufs[j].bitcast(f32)[0:64,
                                                      P + sq * 512:P + (sq + 1) * 512])
                        nc.sync.dma_start(
                            out=pkT_d[2 * j + 1, :, sq * 512:(sq + 1) * 512],
                            in_=kbufs[j].bitcast(f32)[64:128,
                                                      P + sq * 512:P + (sq + 1) * 512])
                # v: natural orientation [s:128, e_v:256]
                for sc in range(4):
                    abs_c = sq * 4 + sc
                    pv_ = ps.tile([128, EV], f32, name=f"pv{abs_c}", tag="scpT",
                                  bufs=2, padded_shape=[128, 1024])
                    for k in range(KCH):
                        nc.tensor.matmul(pv_[:], xbr[k][:, sc * 128:(sc + 1) * 128],
                                         wvr[k][:], start=(k == 0),
                                         stop=(k == KCH - 1))
                    vt = sb.tile([128, EV], f32r, name=f"vt{abs_c}", tag="vt",
                                 bufs=3)
                    if has_bias:
                        nc.vector.tensor_add(out=vt[:], in0=pv_[:], in1=bv_b[:])
                    else:
                        nc.vector.tensor_copy(vt[:], pv_[:])
                    for h in range(NH):
                        nc.sync.dma_start(
                            out=pv_d[h, abs_c * 128:(abs_c + 1) * 128, :],
                            in_=vt.bitcast(f32)[:, h * 64:(h + 1) * 64])
                    for p in range(2):
                        vbv5 = vbufs[p].rearrange("q (i c e) -> q i c e",
                                                  i=2, e=65)
                        nc.vector.tensor_copy(
                            vbv5[:, :, P // 128 + abs_c, 0:64],
                            vt[:, 2 * p * 64:(2 * p + 2) * 64]
                            .rearrange("q (i d) -> q i d", d=64))

            # past-KV loads (prefetched into phase-1 DMA gaps)
            for p in range(2):
                kbuf, vbuf = kbufs[p], vbufs[p]
                st = stg([128, P], f"kstp{p}")
                nc.sync.dma_start(
                    out=st[:],
                    in_=pastKT_d[2 * p:2 * p + 2].rearrange("h d t -> (h d) t"))
                nc.vector.tensor_copy(kbuf[:, 0:P], st[:])
                vbv = vbuf.rearrange("q (c e) -> q c e", e=65)
                for i in range(2):
                    h = 2 * p + i
                    stv = stg([128, P // 128 * 64], f"vstp{p}_{i}")
                    nc.sync.dma_start(
                        out=stv.rearrange("q (c d) -> q c d", d=64),
                        in_=pastV_d[h].rearrange("(c q) d -> q c d", q=128))
                    nc.vector.tensor_copy(
                        vbv[:, i * NTCH:i * NTCH + P // 128, 0:64],
                        stv.rearrange("q (c d) -> q c d", d=64))
                nc.vector.tensor_copy(vbv[:, :, 64:65],
                                      onec.broadcast_to([128, 2 * NTCH, 1]))

            # attention, s-tile outer; AllGather [0:1536] after sn2,
            # AllGather [1536:2048] after sn3
            for sn in range(3):
                for p in range(2):
                    attn(sn, p)
            nc.gpsimd.collective_compute(
                "AllGather",
                mybir.AluOpType.bypass,
                replica_groups=GROUPS,
                ins=[cc_in[0].opt()],
                outs=[cc_out[0].opt()],
            )
            for p in range(2):
                attn(3, p)
            nc.gpsimd.collective_compute(
                "AllGather",
                mybir.AluOpType.bypass,
                replica_groups=GROUPS,
                ins=[cc_in[1].opt()],
                outs=[cc_out[1].opt()],
            )

            # ---- phase 4: output projection, per S half -----------------
            wprs = []
            for k in range(KCH):
                wst = stg([128, OC], f"wpst{k}")
                nc.sync.dma_start(out=wst[:], in_=wproj_d[k * 128:(k + 1) * 128, :])
                wpr = sb.tile([128, OC], f32r, name=f"wpr{k}", tag="wv", bufs=KCH)
                nc.vector.tensor_copy(wpr[:], wst[:])
                wprs.append(wpr)
            for j in range(4):
                w_i = 0 if j < 3 else 1
                col0 = j * 512 if j < 3 else 0
                projp = ps.tile([128, 1024], f32, name=f"pjs{j}",
                                tag="scpT", bufs=2)
                for k in range(KCH):
                    st = stg([128, 512], f"mgst{k}_{j}")
                    d_i = nc.sync.dma_start(
                        out=st[:],
                        in_=cc_out[w_i][k * 128:(k + 1) * 128, col0:col0 + 512])
                    add_dep_helper(d_i.ins, last_insts['dma' + str(w_i)].ins,
                                   sync=False,
                                   reason="phase4 load after phase2 sync stream")
                    mr = sb.tile([128, 512], f32r, name=f"mgr{k}_{j}",
                                 tag="mgr", bufs=2)
                    c_i = nc.vector.tensor_copy(mr[:], st[:])
                    add_dep_helper(c_i.ins, last_insts['dve' + str(w_i)].ins,
                                   sync=False,
                                   reason="phase4 cast after phase2 dve stream")
                    for oc in range(2):
                        nc.tensor.matmul(
                            projp[:, oc * 512:(oc + 1) * 512],
                            wprs[k][:, oc * 128:(oc + 1) * 128],
                            mr[:],
                            start=(k == 0), stop=(k == KCH - 1))
                for oc in range(2):
                    pj = projp[:, oc * 512:(oc + 1) * 512]
                    ost = sb.tile([128, 512], f32, name=f"ost{j}_{oc}",
                                  tag="ost", bufs=3)
                    if has_bias:
                        nc.vector.tensor_scalar_add(ost[:], pj,
                                                    bpr_s[:, oc:oc + 1])
                    else:
                        nc.scalar.copy(out=ost[:], in_=pj)
                    nc.sync.dma_start(
                        out=outT_d[oc * 128:(oc + 1) * 128,
                                   j * 512:(j + 1) * 512],
                        in_=ost[:])

    nc.finalize()
    return nc


def _numpy_fallback(x, mask, past_layer, w_attn, b_attn, w_proj, b_proj):
    qkv = np.einsum("bsd,de->bse", x, w_attn) + b_attn
    q, k, v = np.split(qkv, 3, axis=2)

    def sh(t):
        return t.reshape(B, S, H, DEPTH).transpose(0, 2, 1, 3)

    q, k, v = sh(q), sh(k), sh(v)
    k = np.concatenate([past_layer[:, 0], k], axis=2)
    v = np.concatenate([past_layer[:, 1], v], axis=2)
    present = np.stack([k, v], axis=1)
    scores = np.einsum("bhqd,bhkd->bhqk", q, k) / np.sqrt(np.float32(DEPTH))
    scores = scores + mask * np.float32(-1e9)
    scores = scores - scores.max(axis=-1, keepdims=True)
    e = np.exp(scores)
    attn = e / e.sum(axis=-1, keepdims=True)
    ctx = np.einsum("bhqk,bhkd->bhqd", attn, v)
    merged = ctx.transpose(0, 2, 1, 3).reshape(B, S, D)
    output = np.einsum("bsd,de->bse", merged, w_proj) + b_proj
    return output.astype(np.float32), present.astype(np.float32)


def kernel(x, mask, past_layer, w_attn, b_attn, w_proj, b_proj):
    global LAST_RESULT
    from concourse.bass_utils import run_bass_kernel_spmd

    x = np.asarray(x, dtype=np.float32)
    mask = np.asarray(mask, dtype=np.float32)
    past_layer = np.asarray(past_layer, dtype=np.float32)
    w_attn = np.asarray(w_attn, dtype=np.float32)
    b_attn = np.asarray(b_attn, dtype=np.float32)
    w_proj = np.asarray(w_proj, dtype=np.float32)
    b_proj = np.asarray(b_proj, dtype=np.float32)

    mask2d = np.ascontiguousarray(mask.reshape(S, T))
    mbool = mask2d != 0.0

    # degenerate fully-masked query rows diverge (reference softmax becomes
    # uniform); handle off-device
    if bool(mbool.all(axis=1).any()):
        return _numpy_fallback(x, mask, past_layer, w_attn, b_attn,
                               w_proj, b_proj)

    tri_expect = np.tril(np.ones((128, 128), dtype=np.float32)).T
    diag = 1.0 - mbool[0:128, P:P + 128].T.astype(np.float32)
    if diag.min() == 0.0 and diag.max() == 1.0:
        tri_expect_c = diag
    else:
        tri_expect_c = tri_expect
    plan, mode = _build_plan(mbool, tri_expect_c)

    has_bias = bool(b_attn.any() or b_proj.any())
    key = _plan_key(plan, mode, has_bias)
    if key not in _prog_cache:
        _prog_cache[key] = _build_program(plan, mode, has_bias)
    nc = _prog_cache[key]

    # ---- host-side sharding prep ---------------------------------------
    xT = [np.ascontiguousarray(x[b].T) for b in range(B)]
    in_maps = []
    for c in range(N_CORES):
        b, g = c // 4, c % 4
        hs = list(range(4 * g, 4 * g + 4))
        qcols = np.concatenate([np.arange(64 * h, 64 * h + 64) for h in hs])
        kcols = qcols + D
        vcols = qcols + 2 * D
        m = {
            "xbT": xT[b],
            "wqk": np.ascontiguousarray(
                w_attn[:, np.concatenate([qcols, kcols])]),
            "wv": np.ascontiguousarray(w_attn[:, vcols]),
            "pastKT": np.ascontiguousarray(
                past_layer[b, 0, hs].transpose(0, 2, 1)),
            "pastV": np.ascontiguousarray(past_layer[b, 1, hs]),
            "tri": tri_expect_c,
            "wproj": np.ascontiguousarray(w_proj[:, OC * g:OC * (g + 1)]),
        }
        if has_bias:
            m["bqk"] = np.ascontiguousarray(
                b_attn[np.concatenate([qcols, kcols])])
            m["bv"] = np.ascontiguousarray(b_attn[vcols])
            m["bproj"] = np.ascontiguousarray(b_proj[OC * g:OC * (g + 1)])
        if mode == "general":
            m["multT"] = np.ascontiguousarray(
                (1.0 - mask2d).T.astype(np.float32))
        in_maps.append(m)

    res = run_bass_kernel_spmd(nc, in_maps, list(range(N_CORES)))
    LAST_RESULT = res

    # ---- unshard -------------------------------------------------------
    output = np.empty((B, S, D), dtype=np.float32)
    present = np.empty((B, 2, H, T, DEPTH), dtype=np.float32)
    present[:, 0, :, :P] = past_layer[:, 0]
    present[:, 1, :, :P] = past_layer[:, 1]
    for c in range(N_CORES):
        b, g = c // 4, c % 4
        r = res.results[c]
        output[b, :, OC * g:OC * (g + 1)] = r["outT"].T
        for i, h in enumerate(range(4 * g, 4 * g + 4)):
            present[b, 0, h, P:] = r["pkT"][i].T
            present[b, 1, h, P:] = r["pv"][i]
    return output, present


# revision 16
# speedup vs baseline: 1.0091x; 1.0091x over previous
"""Multi-head attention with KV cache on 8 Trainium2 NeuronCores.

Sharding (per the hint): data-parallel over the batch (2 groups of 4
cores), tensor-parallel over the 16 heads (4 heads per core).  Attention
is fully local per head; the output projection is column-split 4-ways
within each batch group after an AllGather of the per-head contexts.

Layout strategy: everything flows through the PE in "transposed"
orientation so no on-device transposes are needed:
  qkT[e, s] = (w_qk chunks).T @ xT chunks
  scoresT[t, s] = kT-chunk.T @ qT          (softmax free-dim = s)
  ctxT'[d+1, s] = [v | 1].T-chunks @ expT  (row 64 = softmax denominator)
  outT[oc, s] = wproj-chunk.T @ mergedT
v is additionally produced in natural [s, d] orientation (separate
tiling of the same projection) because present-v and the ctx matmul's
stationary operand both want it that way.

Head PAIRS are processed together: scores for the two heads run as
concurrent 64x128 row-tiles of the PE array (tile_position), and one
wide ACTIVATE handles both heads' exp.  All matmuls run in float32r
(TF32-like, full PE rate, ~1.5e-4 scaled error).  The causal mask is
applied structurally: fully-masked t-chunks are skipped and the
diagonal 128x128 block is one multiplicative tri pattern.  The host
verifies the mask really has this structure and falls back to a general
multiplicative-mask path (or numpy for degenerate masks) otherwise.
"""
import os
import sys

sys.path.insert(0, "/opt/trn_rl_repo")

import numpy as np

B, S, D, H, P = 2, 2048, 1024, 16, 2048
T = P + S
DEPTH = D // H          # 64
N_CORES = 8
GROUPS = [[0, 1, 2, 3], [4, 5, 6, 7]]
NH = H // 4             # 4 local heads per core
EQK = 2 * NH * DEPTH    # 512 (q block 256 | k block 256)
EV = NH * DEPTH         # 256
OC = D // 4             # 256 output columns per core
SN = 512                # attention s-tile width
NSN = S // SN           # 4
NTCH = T // 128         # 32 t-chunks
KCH = D // 128          # 8 contraction chunks

_prog_cache = {}
LAST_RESULT = None


def _build_plan(mask2d, tri_expect):
    """Per (s-tile, t-chunk) mask classification.

    plan[sn] = list of (t0, state); state is ('full',), ('tri', off) or
    ('dram',).  Fully-masked chunks are dropped.  mode is 'causal' when
    no 'dram' states exist.
    """
    plan = []
    mode = "causal"
    for sn in range(NSN):
        s0 = sn * SN
        chunks = []
        for t0 in range(0, T, 128):
            sub = mask2d[s0:s0 + SN, t0:t0 + 128]
            if not sub.any():
                chunks.append((t0, ("full",)))
                continue
            if sub.all():
                continue
            st = None
            subT = sub.T  # [t 128, s SN]
            for off in range(0, SN, 128):
                cand = np.ones((128, SN), dtype=np.float32)
                cand[:, :off] = 0.0
                cand[:, off:off + 128] = tri_expect
                if np.array_equal(1.0 - subT, cand):
                    st = ("tri", off)
                    break
            if st is None:
                st = ("dram",)
                mode = "general"
            chunks.append((t0, st))
        plan.append(chunks)
    return plan, mode


def _plan_key(plan, mode, has_bias):
    return (mode, has_bias,
            tuple((sn_i, t0, st)
                  for sn_i, chunks in enumerate(plan)
                  for t0, st in chunks))


def _build_program(plan, mode, has_bias):
    import concourse.bacc as bacc
    import concourse.mybir as mybir
    from concourse import tile as tile_mod
    from concourse.tile import add_dep_helper

    f32 = mybir.dt.float32
    f32r = mybir.dt.float32r
    AF = mybir.ActivationFunctionType

    nc = bacc.Bacc(None)

    # ---- per-core external tensors -------------------------------------
    xbT_d = nc.dram_tensor("xbT", [D, S], f32, kind="ExternalInput")
    wqk_d = nc.dram_tensor("wqk", [D, EQK], f32, kind="ExternalInput")
    wv_d = nc.dram_tensor("wv", [D, EV], f32, kind="ExternalInput")
    pastKT_d = nc.dram_tensor("pastKT", [NH, DEPTH, P], f32, kind="ExternalInput")
    pastV_d = nc.dram_tensor("pastV", [NH, P, DEPTH], f32, kind="ExternalInput")
    tri_d = nc.dram_tensor("tri", [128, 128], f32, kind="ExternalInput")
    wproj_d = nc.dram_tensor("wproj", [D, OC], f32, kind="ExternalInput")
    if has_bias:
        bqk_d = nc.dram_tensor("bqk", [EQK], f32, kind="ExternalInput")
        bv_d = nc.dram_tensor("bv", [EV], f32, kind="ExternalInput")
        bproj_d = nc.dram_tensor("bproj", [OC], f32, kind="ExternalInput")
    if mode == "general":
        multT_d = nc.dram_tensor("multT", [T, S], f32, kind="ExternalInput")

    pkT_d = nc.dram_tensor("pkT", [NH, DEPTH, S], f32, kind="ExternalOutput")
    pv_d = nc.dram_tensor("pv", [NH, S, DEPTH], f32, kind="ExternalOutput")
    outT_d = nc.dram_tensor("outT", [OC, S], f32, kind="ExternalOutput")

    with tile_mod.TileContext(nc) as tc:
        with (
            tc.tile_pool(name="sb", bufs=1) as sb,
            tc.tile_pool(name="ps", bufs=1, space="PSUM") as ps,
            tc.tile_pool(name="dram", bufs=1, space="DRAM") as dram,
        ):
            cc_in = [dram.tile([EV, S // 2], f32, name=f"cc_in{i}") for i in range(2)]
            cc_out = [dram.tile([D, S // 2], f32, name=f"cc_out{i}") for i in range(2)]

            def stg(shape, name):
                return sb.tile(shape, f32, name=name, tag="stg",
                               bufs=4, padded_shape=[128, 2048])

            # ---- small constants ----------------------------------------
            tri_s = stg([128, 128], "tri_s")
            nc.sync.dma_start(out=tri_s[:], in_=tri_d[:])
            tri_r = sb.tile([128, 128], f32r, name="tri_r")
            nc.vector.tensor_copy(tri_r[:], tri_s[:])
            onec = sb.tile([128, 1], f32, name="onec")
            nc.gpsimd.memset(onec[:], 1.0)

            if has_bias:
                bqk_s = sb.tile([128, EQK // 128], f32, name="bqk_s")
                nc.sync.dma_start(out=bqk_s[:],
                                  in_=bqk_d.rearrange("(t p) -> p t", p=128))
                bpr_s = sb.tile([128, OC // 128], f32, name="bpr_s")
                nc.sync.dma_start(out=bpr_s[:],
                                  in_=bproj_d.rearrange("(t p) -> p t", p=128))
                bv_row = sb.tile([1, EV], f32, name="bv_row")
                nc.sync.dma_start(out=bv_row[:], in_=bv_d[None, :])
                bv_b = sb.tile([128, EV], f32, name="bv_b")
                nc.gpsimd.partition_broadcast(bv_b[:], bv_row[:])

            # ---- weights: load + round ----------------------------------
            wqkr = []
            for k in range(KCH):
                st = stg([128, EQK], f"wqkst{k}")
                nc.sync.dma_start(out=st[:], in_=wqk_d[k * 128:(k + 1) * 128, :])
                wr = sb.tile([128, EQK], f32r, name=f"wqkr{k}", tag="wqk", bufs=KCH)
                nc.vector.tensor_copy(wr[:], st[:])
                wqkr.append(wr)
            wvr = []
            for k in range(KCH):
                st = stg([128, EV], f"wvst{k}")
                nc.sync.dma_start(out=st[:], in_=wv_d[k * 128:(k + 1) * 128, :])
                wr = sb.tile([128, EV], f32r, name=f"wvr{k}", tag="wv", bufs=KCH)
                nc.vector.tensor_copy(wr[:], st[:])
                wvr.append(wr)

            # ---- phases 1+2 interleaved by head pair --------------------
            # For each pair: QKV projections for its heads, then its
            # first-half attention (overlaps the other pair's phase 1 /
            # the ACT-bound attention hides the PE/DMA work).
            kbufs, qbufs, vbufs = [None] * 2, [None] * 2, [None] * 2
            last_insts = {}

            def attn(sn, p):
                s0 = sn * SN
                chunks = plan[sn]
                last_ci = len(chunks) - 1
                kbuf, qbuf, vbuf = kbufs[p], qbufs[p], vbufs[p]
                ctxps = [ps.tile([65, SN], f32, name=f"ctx{sn}_{p}_{i}",
                                 tag=("ctxA" if i == 0 else "ctxB"), bufs=2)
                         for i in range(2)]
                for ci, (t0, st_h) in enumerate(chunks):
                    tci = t0 // 128
                    scp = ps.tile([128, 2 * SN], f32, name=f"sc{sn}_{p}_{tci}",
                                  tag="scpT", bufs=2)
                    for i, (r0, tp) in enumerate(((0, (0, 0)), (64, (64, 0)))):
                        nc.tensor.matmul(
                            scp[:, i * SN:(i + 1) * SN],
                            kbuf[r0:r0 + 64, t0:t0 + 128],
                            qbuf[r0:r0 + 64, s0:s0 + SN],
                            start=True, stop=True, tile_position=tp)
                    expt = sb.tile([128, 2 * SN], f32r, name=f"ex{sn}_{p}_{tci}",
                                   tag="expt", bufs=3)
                    nc.scalar.activation(expt[:], scp[:], AF.Exp, scale=0.125)
                    mrr = None
                    for i in range(2):
                        base = i * SN
                        if st_h[0] == "tri":
                            off = st_h[1]
                            if off > 0:
                                nc.vector.tensor_scalar_mul(
                                    expt[:, base:base + off],
                                    expt[:, base:base + off], 0.0)
                            nc.vector.tensor_mul(
                                out=expt[:, base + off:base + off + 128],
                                in0=expt[:, base + off:base + off + 128],
                                in1=tri_r[:])
                        elif st_h[0] == "dram":
                            if mrr is None:
                                mst = stg([128, SN], f"mst{sn}_{p}_{tci}")
                                nc.sync.dma_start(
                                    out=mst[:],
                                    in_=multT_d[t0:t0 + 128, s0:s0 + SN])
                                mrr = sb.tile([128, SN], f32r,
                                              name=f"mrr{sn}_{p}_{tci}",
                                              tag="mrr", bufs=2)
                                nc.vector.tensor_copy(mrr[:], mst[:])
                            nc.vector.tensor_mul(
                                out=expt[:, base:base + SN],
                                in0=expt[:, base:base + SN],
                                in1=mrr[:])
                    for i in range(2):
                        nc.tensor.matmul(
                            ctxps[i][:],
                            vbuf[:, (i * NTCH + tci) * 65:
                                 (i * NTCH + tci) * 65 + 65],
                            expt[:, i * SN:(i + 1) * SN],
                            start=(ci == 0), stop=(ci == last_ci))
                for i in range(2):
                    h = 2 * p + i
                    ctxs = sb.tile([65, SN], f32, name=f"ctxs{sn}_{p}_{i}",
                                   tag="ctxs", bufs=3)
                    nc.vector.tensor_copy(ctxs[:], ctxps[i][:])
                    rec = sb.tile([1, SN], f32, name=f"rec{sn}_{p}_{i}",
                                  tag="rec", bufs=2)
                    nc.vector.reciprocal(out=rec[:], in_=ctxs[64:65, :])
                    recb = sb.tile([64, SN], f32, name=f"recb{sn}_{p}_{i}",
                                   tag="recb", bufs=2)
                    nc.gpsimd.partition_broadcast(recb[:], rec[:])
                    mstg = sb.tile([64, SN], f32, name=f"mstg{sn}_{p}_{i}",
                                   tag="mstg", bufs=3)
                    last_insts['dve'] = nc.vector.tensor_mul(
                        out=mstg[:], in0=ctxs[0:64, :], in1=recb[:])
                    last_insts['dma'] = nc.sync.dma_start(
                        out=cc_in[sn // 2][h * 64:(h + 1) * 64,
                                           (sn % 2) * SN:(sn % 2) * SN + SN],
                        in_=mstg[:])

            for p in range(2):
                kbufs[p] = sb.tile([128, T], f32r, name=f"kbuf{p}", tag="kbuf", bufs=2)
                qbufs[p] = sb.tile([128, S], f32r, name=f"qbuf{p}", tag="qbuf", bufs=2)
                vbufs[p] = sb.tile([128, 2 * NTCH * 65], f32r, name=f"vbuf{p}",
                                   tag="vbuf", bufs=2)

            # ---- phase 1: QKV projections (both pairs) ------------------
            for sq in range(4):         # s quarters of 512
                xbr = []
                for k in range(KCH):
                    st = stg([128, 512], f"xbst{sq}_{k}")
                    nc.sync.dma_start(
                        out=st[:], in_=xbT_d[k * 128:(k + 1) * 128,
                                             sq * 512:(sq + 1) * 512])
                    xr = sb.tile([128, 512], f32r, name=f"xbr{sq}_{k}",
                                 tag="xb", bufs=KCH + 1)
                    nc.vector.tensor_copy(xr[:], st[:])
                    xbr.append(xr)
                for e in range(4):
                    pq = ps.tile([128, 512], f32, name=f"pqk{sq}_{e}",
                                 tag=("ctxA" if e % 2 == 0 else "ctxB"), bufs=2)
                    for k in range(KCH):
                        nc.tensor.matmul(pq[:], wqkr[k][:, e * 128:(e + 1) * 128],
                                         xbr[k][:], start=(k == 0),
                                         stop=(k == KCH - 1))
                    if e < 2:
                        dst = qbufs[e][:, sq * 512:(sq + 1) * 512]
                    else:
                        dst = kbufs[e - 2][:, P + sq * 512:P + (sq + 1) * 512]
                    if has_bias:
                        nc.vector.tensor_scalar_add(dst, pq[:], bqk_s[:, e:e + 1])
                    else:
                        nc.vector.tensor_copy(dst, pq[:])
                    if e >= 2:
                        j = e - 2
                        nc.sync.dma_start(
                            out=pkT_d[2 * j, :, sq * 512:(sq + 1) * 512],
                            in_=kbufs[j].bitcast(f32)[0:64,
                                                      P + sq * 512:P + (sq + 1) * 512])
                        nc.sync.dma_start(
                            out=pkT_d[2 * j + 1, :, sq * 512:(sq + 1) * 512],
                            in_=kbufs[j].bitcast(f32)[64:128,
                                                      P + sq * 512:P + (sq + 1) * 512])
                # v: natural orientation [s:128, e_v:256]
                for sc in range(4):
                    abs_c = sq * 4 + sc
                    pv_ = ps.tile([128, EV], f32, name=f"pv{abs_c}", tag="scpT",
                                  bufs=2, padded_shape=[128, 1024])
                    for k in range(KCH):
                        nc.tensor.matmul(pv_[:], xbr[k][:, sc * 128:(sc + 1) * 128],
                                         wvr[k][:], start=(k == 0),
                                         stop=(k == KCH - 1))
                    vt = sb.tile([128, EV], f32r, name=f"vt{abs_c}", tag="vt",
                                 bufs=3)
                    if has_bias:
                        nc.vector.tensor_add(out=vt[:], in0=pv_[:], in1=bv_b[:])
                    else:
                        nc.vector.tensor_copy(vt[:], pv_[:])
                    for h in range(NH):
                        nc.sync.dma_start(
                            out=pv_d[h, abs_c * 128:(abs_c + 1) * 128, :],
                            in_=vt.bitcast(f32)[:, h * 64:(h + 1) * 64])
                    for p in range(2):
                        vbv5 = vbufs[p].rearrange("q (i c e) -> q i c e",
                                                  i=2, e=65)
                        nc.vector.tensor_copy(
                            vbv5[:, :, P // 128 + abs_c, 0:64],
                            vt[:, 2 * p * 64:(2 * p + 2) * 64]
                            .rearrange("q (i d) -> q i d", d=64))

            # past-KV loads (prefetched into phase-1 DMA gaps)
            for p in range(2):
                kbuf, vbuf = kbufs[p], vbufs[p]
                st = stg([128, P], f"kstp{p}")
                nc.sync.dma_start(
                    out=st[:],
                    in_=pastKT_d[2 * p:2 * p + 2].rearrange("h d t -> (h d) t"))
                nc.vector.tensor_copy(kbuf[:, 0:P], st[:])
                vbv = vbuf.rearrange("q (c e) -> q c e", e=65)
                for i in range(2):
                    h = 2 * p + i
                    stv = stg([128, P // 128 * 64], f"vstp{p}_{i}")
                    nc.sync.dma_start(
                        out=stv.rearrange("q (c d) -> q c d", d=64),
                        in_=pastV_d[h].rearrange("(c q) d -> q c d", q=128))
                    nc.vector.tensor_copy(
                        vbv[:, i * NTCH:i * NTCH + P // 128, 0:64],
                        stv.rearrange("q (c d) -> q c d", d=64))
                nc.vector.tensor_copy(vbv[:, :, 64:65],
                                      onec.broadcast_to([128, 2 * NTCH, 1]))

            # first-half attention, s-tile outer
            for sn in range(2):
                for p in range(2):
                    attn(sn, p)

            # ---- phase 3a: AllGather for the first half of S ------------
            nc.gpsimd.collective_compute(
                "AllGather",
                mybir.AluOpType.bypass,
                replica_groups=GROUPS,
                ins=[cc_in[0].opt()],
                outs=[cc_out[0].opt()],
            )

            # second-half attention
            for sn in range(2, NSN):
                for p in range(2):
                    attn(sn, p)

            # ---- phase 3b: AllGather for the second half of S -----------
            nc.gpsimd.collective_compute(
                "AllGather",
                mybir.AluOpType.bypass,
                replica_groups=GROUPS,
                ins=[cc_in[1].opt()],
                outs=[cc_out[1].opt()],
            )

            # ---- phase 4: output projection, per S half -----------------
            wprs = []
            for k in range(KCH):
                wst = stg([128, OC], f"wpst{k}")
                nc.sync.dma_start(out=wst[:], in_=wproj_d[k * 128:(k + 1) * 128, :])
                wpr = sb.tile([128, OC], f32r, name=f"wpr{k}", tag="wv", bufs=KCH)
                nc.vector.tensor_copy(wpr[:], wst[:])
                wprs.append(wpr)
            for w in range(2):
                projp = [ps.tile([128, 1024], f32, name=f"pj{w}_{oc}",
                                 tag="scpT", bufs=2) for oc in range(2)]
                for k in range(KCH):
                    st = stg([128, 1024], f"mgst{k}_{w}")
                    d_i = nc.sync.dma_start(out=st[:],
                                            in_=cc_out[w][k * 128:(k + 1) * 128, :])
                    add_dep_helper(d_i.ins, last_insts['dma'].ins, sync=False,
                                   reason="phase4 load after phase2 sync stream")
                    mr = sb.tile([128, 1024], f32r, name=f"mgr{k}_{w}",
                                 tag="mgr", bufs=2)
                    c_i = nc.vector.tensor_copy(mr[:], st[:])
                    add_dep_helper(c_i.ins, last_insts['dve'].ins, sync=False,
                                   reason="phase4 cast after phase2 dve stream")
                    for oc in range(2):
                        for s4 in range(2):
                            nc.tensor.matmul(
                                projp[oc][:, s4 * 512:(s4 + 1) * 512],
                                wprs[k][:, oc * 128:(oc + 1) * 128],
                                mr[:, s4 * 512:(s4 + 1) * 512],
                                start=(k == 0), stop=(k == KCH - 1))
                for oc in range(2):
                    for s4 in range(2):
                        pj = projp[oc][:, s4 * 512:(s4 + 1) * 512]
                        ost = sb.tile([128, 512], f32, name=f"ost{w}_{oc}_{s4}",
                                      tag="ost", bufs=3)
                        if has_bias:
                            nc.vector.tensor_scalar_add(ost[:], pj,
                                                        bpr_s[:, oc:oc + 1])
                        else:
                            nc.scalar.copy(out=ost[:], in_=pj)
                        nc.sync.dma_start(
                            out=outT_d[oc * 128:(oc + 1) * 128,
                                       w * 1024 + s4 * 512:w * 1024 + s4 * 512 + 512],
                            in_=ost[:])

    nc.finalize()
    return nc


def _numpy_fallback(x, mask, past_layer, w_attn, b_attn, w_proj, b_proj):
    qkv = np.einsum("bsd,de->bse", x, w_attn) + b_attn
    q, k, v = np.split(qkv, 3, axis=2)

    def sh(t):
        return t.reshape(B, S, H, DEPTH).transpose(0, 2, 1, 3)

    q, k, v = sh(q), sh(k), sh(v)
    k = np.concatenate([past_layer[:, 0], k], axis=2)
    v = np.concatenate([past_layer[:, 1], v], axis=2)
    present = np.stack([k, v], axis=1)
    scores = np.einsum("bhqd,bhkd->bhqk", q, k) / np.sqrt(np.float32(DEPTH))
    scores = scores + mask * np.float32(-1e9)
    scores = scores - scores.max(axis=-1, keepdims=True)
    e = np.exp(scores)
    attn = e / e.sum(axis=-1, keepdims=True)
    ctx = np.einsum("bhqk,bhkd->bhqd", attn, v)
    merged = ctx.transpose(0, 2, 1, 3).reshape(B, S, D)
    output = np.einsum("bsd,de->bse", merged, w_proj) + b_proj
    return output.astype(np.float32), present.astype(np.float32)


def kernel(x, mask, past_layer, w_attn, b_attn, w_proj, b_proj):
    global LAST_RESULT
    from concourse.bass_utils import run_bass_kernel_spmd

    x = np.asarray(x, dtype=np.float32)
    mask = np.asarray(mask, dtype=np.float32)
    past_layer = np.asarray(past_layer, dtype=np.float32)
    w_attn = np.asarray(w_attn, dtype=np.float32)
    b_attn = np.asarray(b_attn, dtype=np.float32)
    w_proj = np.asarray(w_proj, dtype=np.float32)
    b_proj = np.asarray(b_proj, dtype=np.float32)

    mask2d = np.ascontiguousarray(mask.reshape(S, T))
    mbool = mask2d != 0.0

    # degenerate fully-masked query rows diverge (reference softmax becomes
    # uniform); handle off-device
    if bool(mbool.all(axis=1).any()):
        return _numpy_fallback(x, mask, past_layer, w_attn, b_attn,
                               w_proj, b_proj)

    tri_expect = np.tril(np.ones((128, 128), dtype=np.float32)).T
    diag = 1.0 - mbool[0:128, P:P + 128].T.astype(np.float32)
    if diag.min() == 0.0 and diag.max() == 1.0:
        tri_expect_c = diag
    else:
        tri_expect_c = tri_expect
    plan, mode = _build_plan(mbool, tri_expect_c)

    has_bias = bool(b_attn.any() or b_proj.any())
    key = _plan_key(plan, mode, has_bias)
    if key not in _prog_cache:
        _prog_cache[key] = _build_program(plan, mode, has_bias)
    nc = _prog_cache[key]

    # ---- host-side sharding prep ---------------------------------------
    xT = [np.ascontiguousarray(x[b].T) for b in range(B)]
    in_maps = []
    for c in range(N_CORES):
        b, g = c // 4, c % 4
        hs = list(range(4 * g, 4 * g + 4))
        qcols = np.concatenate([np.arange(64 * h, 64 * h + 64) for h in hs])
        kcols = qcols + D
        vcols = qcols + 2 * D
        m = {
            "xbT": xT[b],
            "wqk": np.ascontiguousarray(
                w_attn[:, np.concatenate([qcols, kcols])]),
            "wv": np.ascontiguousarray(w_attn[:, vcols]),
            "pastKT": np.ascontiguousarray(
                past_layer[b, 0, hs].transpose(0, 2, 1)),
            "pastV": np.ascontiguousarray(past_layer[b, 1, hs]),
            "tri": tri_expect_c,
            "wproj": np.ascontiguousarray(w_proj[:, OC * g:OC * (g + 1)]),
        }
        if has_bias:
            m["bqk"] = np.ascontiguousarray(
                b_attn[np.concatenate([qcols, kcols])])
            m["bv"] = np.ascontiguousarray(b_attn[vcols])
            m["bproj"] = np.ascontiguousarray(b_proj[OC * g:OC * (g + 1)])
        if mode == "general":
            m["multT"] = np.ascontiguousarray(
                (1.0 - mask2d).T.astype(np.float32))
        in_maps.append(m)

    res = run_bass_kernel_spmd(nc, in_maps, list(range(N_CORES)))
    LAST_RESULT = res

    # ---- unshard -------------------------------------------------------
    output = np.empty((B, S, D), dtype=np.float32)
    present = np.empty((B, 2, H, T, DEPTH), dtype=np.float32)
    present[:, 0, :, :P] = past_layer[:, 0]
    present[:, 1, :, :P] = past_layer[:, 1]
    for c in range(N_CORES):
        b, g = c // 4, c % 4
        r = res.results[c]
        output[b, :, OC * g:OC * (g + 1)] = r["outT"].T
        for i, h in enumerate(range(4 * g, 4 * g + 4)):
            present[b, 0, h, P:] = r["pkT"][i].T
            present[b, 1, h, P:] = r["pv"][i]
    return output, present


# revision 18
# speedup vs baseline: 1.0167x; 1.0076x over previous
"""Multi-head attention with KV cache on 8 Trainium2 NeuronCores.

Sharding (per the hint): data-parallel over the batch (2 groups of 4
cores), tensor-parallel over the 16 heads (4 heads per core).  Attention
is fully local per head; the output projection is column-split 4-ways
within each batch group after an AllGather of the per-head contexts.

Layout strategy: everything flows through the PE in "transposed"
orientation so no on-device transposes are needed:
  qkT[e, s] = (w_qk chunks).T @ xT chunks
  scoresT[t, s] = kT-chunk.T @ qT          (softmax free-dim = s)
  ctxT'[d+1, s] = [v | 1].T-chunks @ expT  (row 64 = softmax denominator)
  outT[oc, s] = wproj-chunk.T @ mergedT
v is additionally produced in natural [s, d] orientation (separate
tiling of the same projection) because present-v and the ctx matmul's
stationary operand both want it that way.

Head PAIRS are processed together: scores for the two heads run as
concurrent 64x128 row-tiles of the PE array (tile_position), and one
wide ACTIVATE handles both heads' exp.  All matmuls run in float32r
(TF32-like, full PE rate, ~1.5e-4 scaled error).  The causal mask is
applied structurally: fully-masked t-chunks are skipped and the
diagonal 128x128 block is one multiplicative tri pattern.  The host
verifies the mask really has this structure and falls back to a general
multiplicative-mask path (or numpy for degenerate masks) otherwise.
"""
import os
import sys

sys.path.insert(0, "/opt/trn_rl_repo")

import numpy as np

B, S, D, H, P = 2, 2048, 1024, 16, 2048
T = P + S
DEPTH = D // H          # 64
N_CORES = 8
GROUPS = [[0, 1, 2, 3], [4, 5, 6, 7]]
NH = H // 4             # 4 local heads per core
EQK = 2 * NH * DEPTH    # 512 (q block 256 | k block 256)
EV = NH * DEPTH         # 256
OC = D // 4             # 256 output columns per core
SN = 512                # attention s-tile width
NSN = S // SN           # 4
NTCH = T // 128         # 32 t-chunks
KCH = D // 128          # 8 contraction chunks

_prog_cache = {}
LAST_RESULT = None


def _build_plan(mask2d, tri_expect):
    """Per (s-tile, t-chunk) mask classification.

    plan[sn] = list of (t0, state); state is ('full',), ('tri', off) or
    ('dram',).  Fully-masked chunks are dropped.  mode is 'causal' when
    no 'dram' states exist.
    """
    plan = []
    mode = "causal"
    for sn in range(NSN):
        s0 = sn * SN
        chunks = []
        for t0 in range(0, T, 128):
            sub = mask2d[s0:s0 + SN, t0:t0 + 128]
            if not sub.any():
                chunks.append((t0, ("full",)))
                continue
            if sub.all():
                continue
            st = None
            subT = sub.T  # [t 128, s SN]
            for off in range(0, SN, 128):
                cand = np.ones((128, SN), dtype=np.float32)
                cand[:, :off] = 0.0
                cand[:, off:off + 128] = tri_expect
                if np.array_equal(1.0 - subT, cand):
                    st = ("tri", off)
                    break
            if st is None:
                st = ("dram",)
                mode = "general"
            chunks.append((t0, st))
        plan.append(chunks)
    return plan, mode


def _plan_key(plan, mode, has_bias):
    return (mode, has_bias,
            tuple((sn_i, t0, st)
                  for sn_i, chunks in enumerate(plan)
                  for t0, st in chunks))


def _build_program(plan, mode, has_bias):
    import concourse.bacc as bacc
    import concourse.mybir as mybir
    from concourse import tile as tile_mod
    from concourse.tile import add_dep_helper

    f32 = mybir.dt.float32
    f32r = mybir.dt.float32r
    AF = mybir.ActivationFunctionType

    nc = bacc.Bacc(None)

    # ---- per-core external tensors -------------------------------------
    xbT_d = nc.dram_tensor("xbT", [D, S], f32, kind="ExternalInput")
    wqk_d = nc.dram_tensor("wqk", [D, EQK], f32, kind="ExternalInput")
    wv_d = nc.dram_tensor("wv", [D, EV], f32, kind="ExternalInput")
    pastKT_d = nc.dram_tensor("pastKT", [NH, DEPTH, P], f32, kind="ExternalInput")
    pastV_d = nc.dram_tensor("pastV", [NH, P, DEPTH], f32, kind="ExternalInput")
    tri_d = nc.dram_tensor("tri", [128, 128], f32, kind="ExternalInput")
    wproj_d = nc.dram_tensor("wproj", [D, OC], f32, kind="ExternalInput")
    if has_bias:
        bqk_d = nc.dram_tensor("bqk", [EQK], f32, kind="ExternalInput")
        bv_d = nc.dram_tensor("bv", [EV], f32, kind="ExternalInput")
        bproj_d = nc.dram_tensor("bproj", [OC], f32, kind="ExternalInput")
    if mode == "general":
        multT_d = nc.dram_tensor("multT", [T, S], f32, kind="ExternalInput")

    pkT_d = nc.dram_tensor("pkT", [NH, DEPTH, S], f32, kind="ExternalOutput")
    pv_d = nc.dram_tensor("pv", [NH, S, DEPTH], f32, kind="ExternalOutput")
    outT_d = nc.dram_tensor("outT", [OC, S], f32, kind="ExternalOutput")

    with tile_mod.TileContext(nc) as tc:
        with (
            tc.tile_pool(name="sb", bufs=1) as sb,
            tc.tile_pool(name="ps", bufs=1, space="PSUM") as ps,
            tc.tile_pool(name="dram", bufs=1, space="DRAM") as dram,
        ):
            cc_in = [dram.tile([EV, S // 2], f32, name=f"cc_in{i}") for i in range(2)]
            cc_out = [dram.tile([D, S // 2], f32, name=f"cc_out{i}") for i in range(2)]

            def stg(shape, name):
                return sb.tile(shape, f32, name=name, tag="stg",
                               bufs=4, padded_shape=[128, 2048])

            # ---- small constants ----------------------------------------
            tri_s = stg([128, 128], "tri_s")
            nc.sync.dma_start(out=tri_s[:], in_=tri_d[:])
            tri_r = sb.tile([128, 128], f32r, name="tri_r")
            nc.vector.tensor_copy(tri_r[:], tri_s[:])
            onec = sb.tile([128, 1], f32, name="onec")
            nc.gpsimd.memset(onec[:], 1.0)

            if has_bias:
                bqk_s = sb.tile([128, EQK // 128], f32, name="bqk_s")
                nc.sync.dma_start(out=bqk_s[:],
                                  in_=bqk_d.rearrange("(t p) -> p t", p=128))
                bpr_s = sb.tile([128, OC // 128], f32, name="bpr_s")
                nc.sync.dma_start(out=bpr_s[:],
                                  in_=bproj_d.rearrange("(t p) -> p t", p=128))
                bv_row = sb.tile([1, EV], f32, name="bv_row")
                nc.sync.dma_start(out=bv_row[:], in_=bv_d[None, :])
                bv_b = sb.tile([128, EV], f32, name="bv_b")
                nc.gpsimd.partition_broadcast(bv_b[:], bv_row[:])

            # ---- weights: load + round ----------------------------------
            wqkr = []
            for k in range(KCH):
                st = stg([128, EQK], f"wqkst{k}")
                nc.sync.dma_start(out=st[:], in_=wqk_d[k * 128:(k + 1) * 128, :])
                wr = sb.tile([128, EQK], f32r, name=f"wqkr{k}", tag="wqk", bufs=KCH)
                nc.vector.tensor_copy(wr[:], st[:])
                wqkr.append(wr)
            wvr = []
            for k in range(KCH):
                st = stg([128, EV], f"wvst{k}")
                nc.sync.dma_start(out=st[:], in_=wv_d[k * 128:(k + 1) * 128, :])
                wr = sb.tile([128, EV], f32r, name=f"wvr{k}", tag="wv", bufs=KCH)
                nc.vector.tensor_copy(wr[:], st[:])
                wvr.append(wr)

            # ---- phases 1+2 interleaved by head pair --------------------
            # For each pair: QKV projections for its heads, then its
            # first-half attention (overlaps the other pair's phase 1 /
            # the ACT-bound attention hides the PE/DMA work).
            kbufs, qbufs, vbufs = [None] * 2, [None] * 2, [None] * 2
            last_insts = {}

            def attn(sn, p):
                s0 = sn * SN
                chunks = plan[sn]
                last_ci = len(chunks) - 1
                kbuf, qbuf, vbuf = kbufs[p], qbufs[p], vbufs[p]
                ctxps = [ps.tile([65, SN], f32, name=f"ctx{sn}_{p}_{i}",
                                 tag=("ctxA" if i == 0 else "ctxB"), bufs=2)
                         for i in range(2)]
                for ci, (t0, st_h) in enumerate(chunks):
                    tci = t0 // 128
                    scp = ps.tile([128, 2 * SN], f32, name=f"sc{sn}_{p}_{tci}",
                                  tag="scpT", bufs=2)
                    for i, (r0, tp) in enumerate(((0, (0, 0)), (64, (64, 0)))):
                        nc.tensor.matmul(
                            scp[:, i * SN:(i + 1) * SN],
                            kbuf[r0:r0 + 64, t0:t0 + 128],
                            qbuf[r0:r0 + 64, s0:s0 + SN],
                            start=True, stop=True, tile_position=tp)
                    expt = sb.tile([128, 2 * SN], f32r, name=f"ex{sn}_{p}_{tci}",
                                   tag="expt", bufs=3)
                    nc.scalar.activation(expt[:], scp[:], AF.Exp, scale=0.125)
                    mrr = None
                    for i in range(2):
                        base = i * SN
                        if st_h[0] == "tri":
                            off = st_h[1]
                            if off > 0:
                                nc.vector.tensor_scalar_mul(
                                    expt[:, base:base + off],
                                    expt[:, base:base + off], 0.0)
                            nc.vector.tensor_mul(
                                out=expt[:, base + off:base + off + 128],
                                in0=expt[:, base + off:base + off + 128],
                                in1=tri_r[:])
                        elif st_h[0] == "dram":
                            if mrr is None:
                                mst = stg([128, SN], f"mst{sn}_{p}_{tci}")
                                nc.sync.dma_start(
                                    out=mst[:],
                                    in_=multT_d[t0:t0 + 128, s0:s0 + SN])
                                mrr = sb.tile([128, SN], f32r,
                                              name=f"mrr{sn}_{p}_{tci}",
                                              tag="mrr", bufs=2)
                                nc.vector.tensor_copy(mrr[:], mst[:])
                            nc.vector.tensor_mul(
                                out=expt[:, base:base + SN],
                                in0=expt[:, base:base + SN],
                                in1=mrr[:])
                    for i in range(2):
                        nc.tensor.matmul(
                            ctxps[i][:],
                            vbuf[:, (i * NTCH + tci) * 65:
                                 (i * NTCH + tci) * 65 + 65],
                            expt[:, i * SN:(i + 1) * SN],
                            start=(ci == 0), stop=(ci == last_ci))
                for i in range(2):
                    h = 2 * p + i
                    ctxs = sb.tile([65, SN], f32, name=f"ctxs{sn}_{p}_{i}",
                                   tag="ctxs", bufs=3)
                    nc.vector.tensor_copy(ctxs[:], ctxps[i][:])
                    rec = sb.tile([1, SN], f32, name=f"rec{sn}_{p}_{i}",
                                  tag="rec", bufs=2)
                    nc.vector.reciprocal(out=rec[:], in_=ctxs[64:65, :])
                    recb = sb.tile([64, SN], f32, name=f"recb{sn}_{p}_{i}",
                                   tag="recb", bufs=2)
                    nc.gpsimd.partition_broadcast(recb[:], rec[:])
                    mstg = sb.tile([64, SN], f32, name=f"mstg{sn}_{p}_{i}",
                                   tag="mstg", bufs=3)
                    last_insts['dve'] = nc.vector.tensor_mul(
                        out=mstg[:], in0=ctxs[0:64, :], in1=recb[:])
                    last_insts['dma'] = nc.sync.dma_start(
                        out=cc_in[sn // 2][h * 64:(h + 1) * 64,
                                           (sn % 2) * SN:(sn % 2) * SN + SN],
                        in_=mstg[:])

            for p in range(2):
                kbufs[p] = sb.tile([128, T], f32r, name=f"kbuf{p}", tag="kbuf", bufs=2)
                qbufs[p] = sb.tile([128, S], f32r, name=f"qbuf{p}", tag="qbuf", bufs=2)
                vbufs[p] = sb.tile([128, 2 * NTCH * 65], f32r, name=f"vbuf{p}",
                                   tag="vbuf", bufs=2)

            # ---- phase 1: QKV projections (both pairs) ------------------
            for sq in range(4):         # s quarters of 512
                xbr = []
                for k in range(KCH):
                    st = stg([128, 512], f"xbst{sq}_{k}")
                    nc.sync.dma_start(
                        out=st[:], in_=xbT_d[k * 128:(k + 1) * 128,
                                             sq * 512:(sq + 1) * 512])
                    xr = sb.tile([128, 512], f32r, name=f"xbr{sq}_{k}",
                                 tag="xb", bufs=KCH + 1)
                    nc.vector.tensor_copy(xr[:], st[:])
                    xbr.append(xr)
                for e in range(4):
                    pq = ps.tile([128, 512], f32, name=f"pqk{sq}_{e}",
                                 tag=("ctxA" if e % 2 == 0 else "ctxB"), bufs=2)
                    for k in range(KCH):
                        nc.tensor.matmul(pq[:], wqkr[k][:, e * 128:(e + 1) * 128],
                                         xbr[k][:], start=(k == 0),
                                         stop=(k == KCH - 1))
                    if e < 2:
                        dst = qbufs[e][:, sq * 512:(sq + 1) * 512]
                    else:
                        dst = kbufs[e - 2][:, P + sq * 512:P + (sq + 1) * 512]
                    if has_bias:
                        nc.vector.tensor_scalar_add(dst, pq[:], bqk_s[:, e:e + 1])
                    else:
                        nc.vector.tensor_copy(dst, pq[:])
                    if e >= 2:
                        j = e - 2
                        nc.sync.dma_start(
                            out=pkT_d[2 * j, :, sq * 512:(sq + 1) * 512],
                            in_=kbufs[j].bitcast(f32)[0:64,
                                                      P + sq * 512:P + (sq + 1) * 512])
                        nc.sync.dma_start(
                            out=pkT_d[2 * j + 1, :, sq * 512:(sq + 1) * 512],
                            in_=kbufs[j].bitcast(f32)[64:128,
                                                      P + sq * 512:P + (sq + 1) * 512])
                # v: natural orientation [s:128, e_v:256]
                for sc in range(4):
                    abs_c = sq * 4 + sc
                    pv_ = ps.tile([128, EV], f32, name=f"pv{abs_c}", tag="scpT",
                                  bufs=2, padded_shape=[128, 1024])
                    for k in range(KCH):
                        nc.tensor.matmul(pv_[:], xbr[k][:, sc * 128:(sc + 1) * 128],
                                         wvr[k][:], start=(k == 0),
                                         stop=(k == KCH - 1))
                    vt = sb.tile([128, EV], f32r, name=f"vt{abs_c}", tag="vt",
                                 bufs=3)
                    if has_bias:
                        nc.vector.tensor_add(out=vt[:], in0=pv_[:], in1=bv_b[:])
                    else:
                        nc.vector.tensor_copy(vt[:], pv_[:])
                    for h in range(NH):
                        nc.sync.dma_start(
                            out=pv_d[h, abs_c * 128:(abs_c + 1) * 128, :],
                            in_=vt.bitcast(f32)[:, h * 64:(h + 1) * 64])
                    for p in range(2):
                        vbv5 = vbufs[p].rearrange("q (i c e) -> q i c e",
                                                  i=2, e=65)
                        nc.vector.tensor_copy(
                            vbv5[:, :, P // 128 + abs_c, 0:64],
                            vt[:, 2 * p * 64:(2 * p + 2) * 64]
                            .rearrange("q (i d) -> q i d", d=64))

            # past-KV loads (prefetched into phase-1 DMA gaps)
            for p in range(2):
                kbuf, vbuf = kbufs[p], vbufs[p]
                st = stg([128, P], f"kstp{p}")
                nc.sync.dma_start(
                    out=st[:],
                    in_=pastKT_d[2 * p:2 * p + 2].rearrange("h d t -> (h d) t"))
                nc.vector.tensor_copy(kbuf[:, 0:P], st[:])
                vbv = vbuf.rearrange("q (c e) -> q c e", e=65)
                for i in range(2):
                    h = 2 * p + i
                    stv = stg([128, P // 128 * 64], f"vstp{p}_{i}")
                    nc.sync.dma_start(
                        out=stv.rearrange("q (c d) -> q c d", d=64),
                        in_=pastV_d[h].rearrange("(c q) d -> q c d", q=128))
                    nc.vector.tensor_copy(
                        vbv[:, i * NTCH:i * NTCH + P // 128, 0:64],
                        stv.rearrange("q (c d) -> q c d", d=64))
                nc.vector.tensor_copy(vbv[:, :, 64:65],
                                      onec.broadcast_to([128, 2 * NTCH, 1]))

            # first-half attention, s-tile outer
            for sn in range(2):
                for p in range(2):
                    attn(sn, p)

            # ---- phase 3a: AllGather for the first half of S ------------
            nc.gpsimd.collective_compute(
                "AllGather",
                mybir.AluOpType.bypass,
                replica_groups=GROUPS,
                ins=[cc_in[0].opt()],
                outs=[cc_out[0].opt()],
            )

            # second-half attention
            for sn in range(2, NSN):
                for p in range(2):
                    attn(sn, p)

            # ---- phase 3b: AllGather for the second half of S -----------
            nc.gpsimd.collective_compute(
                "AllGather",
                mybir.AluOpType.bypass,
                replica_groups=GROUPS,
                ins=[cc_in[1].opt()],
                outs=[cc_out[1].opt()],
            )

            # ---- phase 4: output projection, per S half -----------------
            wprs = []
            for k in range(KCH):
                wst = stg([128, OC], f"wpst{k}")
                nc.sync.dma_start(out=wst[:], in_=wproj_d[k * 128:(k + 1) * 128, :])
                wpr = sb.tile([128, OC], f32r, name=f"wpr{k}", tag="wv", bufs=KCH)
                nc.vector.tensor_copy(wpr[:], wst[:])
                wprs.append(wpr)
            for w in range(2):
                projp = [ps.tile([128, 1024], f32, name=f"pj{w}_{oc}",
                                 tag="scpT", bufs=2) for oc in range(2)]
                for k in range(KCH):
                    st = stg([128, 1024], f"mgst{k}_{w}")
                    d_i = nc.sync.dma_start(out=st[:],
                                            in_=cc_out[w][k * 128:(k + 1) * 128, :])
                    add_dep_helper(d_i.ins, last_insts['dma'].ins, sync=False,
                                   reason="phase4 load after phase2 sync stream")
                    mr = sb.tile([128, 1024], f32r, name=f"mgr{k}_{w}",
                                 tag="mgr", bufs=2)
                    c_i = nc.vector.tensor_copy(mr[:], st[:])
                    add_dep_helper(c_i.ins, last_insts['dve'].ins, sync=False,
                                   reason="phase4 cast after phase2 dve stream")
                    for oc in range(2):
                        for s4 in range(2):
                            nc.tensor.matmul(
                                projp[oc][:, s4 * 512:(s4 + 1) * 512],
                                wprs[k][:, oc * 128:(oc + 1) * 128],
                                mr[:, s4 * 512:(s4 + 1) * 512],
                                start=(k == 0), stop=(k == KCH - 1))
                for oc in range(2):
                    for s4 in range(2):
                        pj = projp[oc][:, s4 * 512:(s4 + 1) * 512]
                        ost = sb.tile([128, 512], f32, name=f"ost{w}_{oc}_{s4}",
                                      tag="ost", bufs=3)
                        if has_bias:
                            nc.vector.tensor_scalar_add(ost[:], pj,
                                                        bpr_s[:, oc:oc + 1])
                        else:
                            nc.scalar.copy(out=ost[:], in_=pj)
                        nc.sync.dma_start(
                            out=outT_d[oc * 128:(oc + 1) * 128,
                                       w * 1024 + s4 * 512:w * 1024 + s4 * 512 + 512],
                            in_=ost[:])

    nc.finalize()
    return nc


def _numpy_fallback(x, mask, past_layer, w_attn, b_attn, w_proj, b_proj):
    qkv = np.einsum("bsd,de->bse", x, w_attn) + b_attn
    q, k, v = np.split(qkv, 3, axis=2)

    def sh(t):
        return t.reshape(B, S, H, DEPTH).transpose(0, 2, 1, 3)

    q, k, v = sh(q), sh(k), sh(v)
    k = np.concatenate([past_layer[:, 0], k], axis=2)
    v = np.concatenate([past_layer[:, 1], v], axis=2)
    present = np.stack([k, v], axis=1)
    scores = np.einsum("bhqd,bhkd->bhqk", q, k) / np.sqrt(np.float32(DEPTH))
    scores = scores + mask * np.float32(-1e9)
    scores = scores - scores.max(axis=-1, keepdims=True)
    e = np.exp(scores)
    attn = e / e.sum(axis=-1, keepdims=True)
    ctx = np.einsum("bhqk,bhkd->bhqd", attn, v)
    merged = ctx.transpose(0, 2, 1, 3).reshape(B, S, D)
    output = np.einsum("bsd,de->bse", merged, w_proj) + b_proj
    return output.astype(np.float32), present.astype(np.float32)


def kernel(x, mask, past_layer, w_attn, b_attn, w_proj, b_proj):
    global LAST_RESULT
    from concourse.bass_utils import run_bass_kernel_spmd

    x = np.asarray(x, dtype=np.float32)
    mask = np.asarray(mask, dtype=np.float32)
    past_layer = np.asarray(past_layer, dtype=np.float32)
    w_attn = np.asarray(w_attn, dtype=np.float32)
    b_attn = np.asarray(b_attn, dtype=np.float32)
    w_proj = np.asarray(w_proj, dtype=np.float32)
    b_proj = np.asarray(b_proj, dtype=np.float32)

    mask2d = np.ascontiguousarray(mask.reshape(S, T))
    mbool = mask2d != 0.0

    # degenerate fully-masked query rows diverge (reference softmax becomes
    # uniform); handle off-device
    if bool(mbool.all(axis=1).any()):
        return _numpy_fallback(x, mask, past_layer, w_attn, b_attn,
                               w_proj, b_proj)

    tri_expect = np.tril(np.ones((128, 128), dtype=np.float32)).T
    diag = 1.0 - mbool[0:128, P:P + 128].T.astype(np.float32)
    if diag.min() == 0.0 and diag.max() == 1.0:
        tri_expect_c = diag
    else:
        tri_expect_c = tri_expect
    plan, mode = _build_plan(mbool, tri_expect_c)

    has_bias = bool(b_attn.any() or b_proj.any())
    key = _plan_key(plan, mode, has_bias)
    if key not in _prog_cache:
        _prog_cache[key] = _build_program(plan, mode, has_bias)
    nc = _prog_cache[key]

    # ---- host-side sharding prep ---------------------------------------
    xT = [np.ascontiguousarray(x[b].T) for b in range(B)]
    in_maps = []
    for c in range(N_CORES):
        b, g = c // 4, c % 4
        hs = list(range(4 * g, 4 * g + 4))
        qcols = np.concatenate([np.arange(64 * h, 64 * h + 64) for h in hs])
        kcols = qcols + D
        vcols = qcols + 2 * D
        m = {
            "xbT": xT[b],
            "wqk": np.ascontiguousarray(
                w_attn[:, np.concatenate([qcols, kcols])]),
            "wv": np.ascontiguousarray(w_attn[:, vcols]),
            "pastKT": np.ascontiguousarray(
                past_layer[b, 0, hs].transpose(0, 2, 1)),
            "pastV": np.ascontiguousarray(past_layer[b, 1, hs]),
            "tri": tri_expect_c,
            "wproj": np.ascontiguousarray(w_proj[:, OC * g:OC * (g + 1)]),
        }
        if has_bias:
            m["bqk"] = np.ascontiguousarray(
                b_attn[np.concatenate([qcols, kcols])])
            m["bv"] = np.ascontiguousarray(b_attn[vcols])
            m["bproj"] = np.ascontiguousarray(b_proj[OC * g:OC * (g + 1)])
        if mode == "general":
            m["multT"] = np.ascontiguousarray(
                (1.0 - mask2d).T.astype(np.float32))
        in_maps.append(m)

    res = run_bass_kernel_spmd(nc, in_maps, list(range(N_CORES)))
    LAST_RESULT = res

    # ---- unshard -------------------------------------------------------
    output = np.empty((B, S, D), dtype=np.float32)
    present = np.empty((B, 2, H, T, DEPTH), dtype=np.float32)
    present[:, 0, :, :P] = past_layer[:, 0]
    present[:, 1, :, :P] = past_layer[:, 1]
    for c in range(N_CORES):
        b, g = c // 4, c % 4
        r = res.results[c]
        output[b, :, OC * g:OC * (g + 1)] = r["outT"].T
        for i, h in enumerate(range(4 * g, 4 * g + 4)):
            present[b, 0, h, P:] = r["pkT"][i].T
            present[b, 1, h, P:] = r["pv"][i]
    return output, present


# revision 20
# speedup vs baseline: 1.2190x; 1.1989x over previous
"""Multi-head attention with KV cache on 8 Trainium2 NeuronCores.

Sharding (per the hint): data-parallel over the batch (2 groups of 4
cores), tensor-parallel over the 16 heads (4 heads per core).  Attention
is fully local per head; the output projection is column-split 4-ways
within each batch group after an AllGather of the per-head contexts.

Layout strategy: everything flows through the PE in "transposed"
orientation so no on-device transposes are needed:
  qkT[e, s] = (w_qk chunks).T @ xT chunks
  scoresT[t, s] = kT-chunk.T @ qT          (softmax free-dim = s)
  ctxT'[d+1, s] = [v | 1].T-chunks @ expT  (row 64 = softmax denominator)
  outT[oc, s] = wproj-chunk.T @ mergedT
v is additionally produced in natural [s, d] orientation (separate
tiling of the same projection) because present-v and the ctx matmul's
stationary operand both want it that way.

Head PAIRS are processed together: scores for the two heads run as
concurrent 64x128 row-tiles of the PE array (tile_position), and one
wide ACTIVATE handles both heads' exp.  All matmuls run in float32r
(TF32-like, full PE rate, ~1.5e-4 scaled error).  The causal mask is
applied structurally: fully-masked t-chunks are skipped and the
diagonal 128x128 block is one multiplicative tri pattern.  The host
verifies the mask really has this structure and falls back to a general
multiplicative-mask path (or numpy for degenerate masks) otherwise.
"""
import os
import sys

sys.path.insert(0, "/opt/trn_rl_repo")

import numpy as np

B, S, D, H, P = 2, 2048, 1024, 16, 2048
T = P + S
DEPTH = D // H          # 64
N_CORES = 8
GROUPS = [[0, 1, 2, 3], [4, 5, 6, 7]]
NH = H // 4             # 4 local heads per core
EQK = 2 * NH * DEPTH    # 512 (q block 256 | k block 256)
EV = NH * DEPTH         # 256
OC = D // 4             # 256 output columns per core
SN = 512                # attention s-tile width
NSN = S // SN           # 4
NTCH = T // 128         # 32 t-chunks
KCH = D // 128          # 8 contraction chunks

_prog_cache = {}
LAST_RESULT = None


def _build_plan(mask2d, tri_expect):
    """Per (s-tile, t-chunk) mask classification.

    plan[sn] = list of (t0, state); state is ('full',), ('tri', off) or
    ('dram',).  Fully-masked chunks are dropped.  mode is 'causal' when
    no 'dram' states exist.
    """
    plan = []
    mode = "causal"
    for sn in range(NSN):
        s0 = sn * SN
        chunks = []
        for t0 in range(0, T, 128):
            sub = mask2d[s0:s0 + SN, t0:t0 + 128]
            if not sub.any():
                chunks.append((t0, ("full",)))
                continue
            if sub.all():
                continue
            st = None
            subT = sub.T  # [t 128, s SN]
            for off in range(0, SN, 128):
                cand = np.ones((128, SN), dtype=np.float32)
                cand[:, :off] = 0.0
                cand[:, off:off + 128] = tri_expect
                if np.array_equal(1.0 - subT, cand):
                    st = ("tri", off)
                    break
            if st is None:
                st = ("dram",)
                mode = "general"
            chunks.append((t0, st))
        plan.append(chunks)
    return plan, mode


def _plan_key(plan, mode, has_bias):
    return (mode, has_bias,
            tuple((sn_i, t0, st)
                  for sn_i, chunks in enumerate(plan)
                  for t0, st in chunks))


def _build_program(plan, mode, has_bias):
    import concourse.bacc as bacc
    import concourse.mybir as mybir
    from concourse import tile as tile_mod
    from concourse.tile import add_dep_helper

    f32 = mybir.dt.float32
    f32r = mybir.dt.float32r
    AF = mybir.ActivationFunctionType

    nc = bacc.Bacc(None)

    # ---- per-core external tensors -------------------------------------
    xbT_d = nc.dram_tensor("xbT", [D, S], f32, kind="ExternalInput")
    wqk_d = nc.dram_tensor("wqk", [D, EQK], f32, kind="ExternalInput")
    wv_d = nc.dram_tensor("wv", [D, EV], f32, kind="ExternalInput")
    pastKT_d = nc.dram_tensor("pastKT", [NH, DEPTH, P], f32, kind="ExternalInput")
    pastV_d = nc.dram_tensor("pastV", [NH, P, DEPTH], f32, kind="ExternalInput")
    tri_d = nc.dram_tensor("tri", [128, 128], f32, kind="ExternalInput")
    wproj_d = nc.dram_tensor("wproj", [D, OC], f32, kind="ExternalInput")
    if has_bias:
        bqk_d = nc.dram_tensor("bqk", [EQK], f32, kind="ExternalInput")
        bv_d = nc.dram_tensor("bv", [EV], f32, kind="ExternalInput")
        bproj_d = nc.dram_tensor("bproj", [OC], f32, kind="ExternalInput")
    if mode == "general":
        multT_d = nc.dram_tensor("multT", [T, S], f32, kind="ExternalInput")

    pkT_d = nc.dram_tensor("pkT", [NH, DEPTH, S], f32, kind="ExternalOutput")
    pv_d = nc.dram_tensor("pv", [NH, S, DEPTH], f32, kind="ExternalOutput")
    outT_d = nc.dram_tensor("outT", [OC, S], f32, kind="ExternalOutput")

    with tile_mod.TileContext(nc) as tc:
        with (
            tc.tile_pool(name="sb", bufs=1) as sb,
            tc.tile_pool(name="ps", bufs=1, space="PSUM") as ps,
            tc.tile_pool(name="dram", bufs=1, space="DRAM") as dram,
        ):
            cc_in = [dram.tile([EV, SN], f32, name=f"cc_in{i}") for i in range(NSN)]
            cc_out = [dram.tile([D, SN], f32, name=f"cc_out{i}") for i in range(NSN)]

            def stg(shape, name):
                return sb.tile(shape, f32, name=name, tag="stg",
                               bufs=4, padded_shape=[128, 2048])

            # ---- small constants ----------------------------------------
            tri_s = stg([128, 128], "tri_s")
            nc.sync.dma_start(out=tri_s[:], in_=tri_d[:])
            tri_r = sb.tile([128, 128], f32r, name="tri_r")
            nc.vector.tensor_copy(tri_r[:], tri_s[:])
            onec = sb.tile([128, 1], f32, name="onec")
            nc.gpsimd.memset(onec[:], 1.0)

            if has_bias:
                bqk_s = sb.tile([128, EQK // 128], f32, name="bqk_s")
                nc.sync.dma_start(out=bqk_s[:],
                                  in_=bqk_d.rearrange("(t p) -> p t", p=128))
                bpr_s = sb.tile([128, OC // 128], f32, name="bpr_s")
                nc.sync.dma_start(out=bpr_s[:],
                                  in_=bproj_d.rearrange("(t p) -> p t", p=128))
                bv_row = sb.tile([1, EV], f32, name="bv_row")
                nc.sync.dma_start(out=bv_row[:], in_=bv_d[None, :])
                bv_b = sb.tile([128, EV], f32, name="bv_b")
                nc.gpsimd.partition_broadcast(bv_b[:], bv_row[:])

            # ---- weights: load + round ----------------------------------
            wqkr = []
            for k in range(KCH):
                st = stg([128, EQK], f"wqkst{k}")
                nc.sync.dma_start(out=st[:], in_=wqk_d[k * 128:(k + 1) * 128, :])
                wr = sb.tile([128, EQK], f32r, name=f"wqkr{k}", tag="wqk", bufs=KCH)
                nc.vector.tensor_copy(wr[:], st[:])
                wqkr.append(wr)
            wvr = []
            for k in range(KCH):
                st = stg([128, EV], f"wvst{k}")
                nc.sync.dma_start(out=st[:], in_=wv_d[k * 128:(k + 1) * 128, :])
                wr = sb.tile([128, EV], f32r, name=f"wvr{k}", tag="wv", bufs=KCH)
                nc.vector.tensor_copy(wr[:], st[:])
                wvr.append(wr)

            # ---- phases 1+2 interleaved by head pair --------------------
            # For each pair: QKV projections for its heads, then its
            # first-half attention (overlaps the other pair's phase 1 /
            # the ACT-bound attention hides the PE/DMA work).
            kbufs, qbufs, vbufs = [None] * 2, [None] * 2, [None] * 2
            last_insts = {}

            def attn(sn, p):
                s0 = sn * SN
                chunks = plan[sn]
                last_ci = len(chunks) - 1
                kbuf, qbuf, vbuf = kbufs[p], qbufs[p], vbufs[p]
                ctxps = [ps.tile([65, SN], f32, name=f"ctx{sn}_{p}_{i}",
                                 tag=("ctxA" if i == 0 else "ctxB"), bufs=2)
                         for i in range(2)]
                for ci, (t0, st_h) in enumerate(chunks):
                    tci = t0 // 128
                    scp = ps.tile([128, 2 * SN], f32, name=f"sc{sn}_{p}_{tci}",
                                  tag="scpT", bufs=2)
                    for i, (r0, tp) in enumerate(((0, (0, 0)), (64, (64, 0)))):
                        nc.tensor.matmul(
                            scp[:, i * SN:(i + 1) * SN],
                            kbuf[r0:r0 + 64, t0:t0 + 128],
                            qbuf[r0:r0 + 64, s0:s0 + SN],
                            start=True, stop=True, tile_position=tp)
                    expt = sb.tile([128, 2 * SN], f32r, name=f"ex{sn}_{p}_{tci}",
                                   tag="expt", bufs=3)
                    nc.scalar.activation(expt[:], scp[:], AF.Exp, scale=0.125)
                    mrr = None
                    for i in range(2):
                        base = i * SN
                        if st_h[0] == "tri":
                            off = st_h[1]
                            if off > 0:
                                nc.vector.tensor_scalar_mul(
                                    expt[:, base:base + off],
                                    expt[:, base:base + off], 0.0)
                            nc.vector.tensor_mul(
                                out=expt[:, base + off:base + off + 128],
                                in0=expt[:, base + off:base + off + 128],
                                in1=tri_r[:])
                        elif st_h[0] == "dram":
                            if mrr is None:
                                mst = stg([128, SN], f"mst{sn}_{p}_{tci}")
                                nc.sync.dma_start(
                                    out=mst[:],
                                    in_=multT_d[t0:t0 + 128, s0:s0 + SN])
                                mrr = sb.tile([128, SN], f32r,
                                              name=f"mrr{sn}_{p}_{tci}",
                                              tag="mrr", bufs=2)
                                nc.vector.tensor_copy(mrr[:], mst[:])
                            nc.vector.tensor_mul(
                                out=expt[:, base:base + SN],
                                in0=expt[:, base:base + SN],
                                in1=mrr[:])
                    for i in range(2):
                        nc.tensor.matmul(
                            ctxps[i][:],
                            vbuf[:, (i * NTCH + tci) * 65:
                                 (i * NTCH + tci) * 65 + 65],
                            expt[:, i * SN:(i + 1) * SN],
                            start=(ci == 0), stop=(ci == last_ci))
                for i in range(2):
                    h = 2 * p + i
                    ctxs = sb.tile([65, SN], f32, name=f"ctxs{sn}_{p}_{i}",
                                   tag="ctxs", bufs=3)
                    nc.vector.tensor_copy(ctxs[:], ctxps[i][:])
                    rec = sb.tile([1, SN], f32, name=f"rec{sn}_{p}_{i}",
                                  tag="rec", bufs=2)
                    nc.vector.reciprocal(out=rec[:], in_=ctxs[64:65, :])
                    recb = sb.tile([64, SN], f32, name=f"recb{sn}_{p}_{i}",
                                   tag="recb", bufs=2)
                    nc.gpsimd.partition_broadcast(recb[:], rec[:])
                    mstg = sb.tile([64, SN], f32, name=f"mstg{sn}_{p}_{i}",
                                   tag="mstg", bufs=3)
                    last_insts['dve%d' % sn] = nc.vector.tensor_mul(
                        out=mstg[:], in0=ctxs[0:64, :], in1=recb[:])
                    last_insts['dma%d' % sn] = nc.sync.dma_start(
                        out=cc_in[sn][h * 64:(h + 1) * 64, :],
                        in_=mstg[:])

            for p in range(2):
                kbufs[p] = sb.tile([128, T], f32r, name=f"kbuf{p}", tag="kbuf", bufs=2)
                qbufs[p] = sb.tile([128, S], f32r, name=f"qbuf{p}", tag="qbuf", bufs=2)
                vbufs[p] = sb.tile([128, 2 * NTCH * 65], f32r, name=f"vbuf{p}",
                                   tag="vbuf", bufs=2)

            # ---- phase 1: QKV projections (both pairs) ------------------
            for sq in range(4):         # s quarters of 512
                xbr = []
                for k in range(KCH):
                    st = stg([128, 512], f"xbst{sq}_{k}")
                    nc.sync.dma_start(
                        out=st[:], in_=xbT_d[k * 128:(k + 1) * 128,
                                             sq * 512:(sq + 1) * 512])
                    xr = sb.tile([128, 512], f32r, name=f"xbr{sq}_{k}",
                                 tag="xb", bufs=KCH + 1)
                    nc.vector.tensor_copy(xr[:], st[:])
                    xbr.append(xr)
                for e in range(4):
                    pq = ps.tile([128, 512], f32, name=f"pqk{sq}_{e}",
                                 tag=("ctxA" if e % 2 == 0 else "ctxB"), bufs=2)
                    for k in range(KCH):
                        nc.tensor.matmul(pq[:], wqkr[k][:, e * 128:(e + 1) * 128],
                                         xbr[k][:], start=(k == 0),
                                         stop=(k == KCH - 1))
                    if e < 2:
                        dst = qbufs[e][:, sq * 512:(sq + 1) * 512]
                    else:
                        dst = kbufs[e - 2][:, P + sq * 512:P + (sq + 1) * 512]
                    if has_bias:
                        nc.vector.tensor_scalar_add(dst, pq[:], bqk_s[:, e:e + 1])
                    else:
                        nc.vector.tensor_copy(dst, pq[:])
                    if e >= 2:
                        j = e - 2
                        nc.sync.dma_start(
                            out=pkT_d[2 * j, :, sq * 512:(sq + 1) * 512],
                            in_=kbufs[j].bitcast(f32)[0:64,
                                                      P + sq * 512:P + (sq + 1) * 512])
                        nc.sync.dma_start(
                            out=pkT_d[2 * j + 1, :, sq * 512:(sq + 1) * 512],
                            in_=kbufs[j].bitcast(f32)[64:128,
                                                      P + sq * 512:P + (sq + 1) * 512])
                # v: natural orientation [s:128, e_v:256]
                for sc in range(4):
                    abs_c = sq * 4 + sc
                    pv_ = ps.tile([128, EV], f32, name=f"pv{abs_c}", tag="scpT",
                                  bufs=2, padded_shape=[128, 1024])
                    for k in range(KCH):
                        nc.tensor.matmul(pv_[:], xbr[k][:, sc * 128:(sc + 1) * 128],
                                         wvr[k][:], start=(k == 0),
                                         stop=(k == KCH - 1))
                    vt = sb.tile([128, EV], f32r, name=f"vt{abs_c}", tag="vt",
                                 bufs=3)
                    if has_bias:
                        nc.vector.tensor_add(out=vt[:], in0=pv_[:], in1=bv_b[:])
                    else:
                        nc.vector.tensor_copy(vt[:], pv_[:])
                    for h in range(NH):
                        nc.sync.dma_start(
                            out=pv_d[h, abs_c * 128:(abs_c + 1) * 128, :],
                            in_=vt.bitcast(f32)[:, h * 64:(h + 1) * 64])
                    for p in range(2):
                        vbv5 = vbufs[p].rearrange("q (i c e) -> q i c e",
                                                  i=2, e=65)
                        nc.vector.tensor_copy(
                            vbv5[:, :, P // 128 + abs_c, 0:64],
                            vt[:, 2 * p * 64:(2 * p + 2) * 64]
                            .rearrange("q (i d) -> q i d", d=64))

            # past-KV loads (prefetched into phase-1 DMA gaps)
            for p in range(2):
                kbuf, vbuf = kbufs[p], vbufs[p]
                st = stg([128, P], f"kstp{p}")
                nc.sync.dma_start(
                    out=st[:],
                    in_=pastKT_d[2 * p:2 * p + 2].rearrange("h d t -> (h d) t"))
                nc.vector.tensor_copy(kbuf[:, 0:P], st[:])
                vbv = vbuf.rearrange("q (c e) -> q c e", e=65)
                for i in range(2):
                    h = 2 * p + i
                    stv = stg([128, P // 128 * 64], f"vstp{p}_{i}")
                    nc.sync.dma_start(
                        out=stv.rearrange("q (c d) -> q c d", d=64),
                        in_=pastV_d[h].rearrange("(c q) d -> q c d", q=128))
                    nc.vector.tensor_copy(
                        vbv[:, i * NTCH:i * NTCH + P // 128, 0:64],
                        stv.rearrange("q (c d) -> q c d", d=64))
                nc.vector.tensor_copy(vbv[:, :, 64:65],
                                      onec.broadcast_to([128, 2 * NTCH, 1]))

            # attention s-tile outer; AllGather fired per s-tile
            for sn in range(NSN):
                for p in range(2):
                    attn(sn, p)
                nc.gpsimd.collective_compute(
                    "AllGather",
                    mybir.AluOpType.bypass,
                    replica_groups=GROUPS,
                    ins=[cc_in[sn].opt()],
                    outs=[cc_out[sn].opt()],
                )

            # ---- phase 4: output projection, per S half -----------------
            wprs = []
            for k in range(KCH):
                wst = stg([128, OC], f"wpst{k}")
                nc.sync.dma_start(out=wst[:], in_=wproj_d[k * 128:(k + 1) * 128, :])
                wpr = sb.tile([128, OC], f32r, name=f"wpr{k}", tag="wv", bufs=KCH)
                nc.vector.tensor_copy(wpr[:], wst[:])
                wprs.append(wpr)
            for j in range(NSN):
                projp = ps.tile([128, 1024], f32, name=f"pjs{j}",
                                tag="scpT", bufs=2)
                for k in range(KCH):
                    st = stg([128, SN], f"mgst{k}_{j}")
                    d_i = nc.sync.dma_start(
                        out=st[:], in_=cc_out[j][k * 128:(k + 1) * 128, :])
                    add_dep_helper(d_i.ins, last_insts['dma%d' % min(j + 1, NSN - 1)].ins,
                                   sync=False,
                                   reason="phase4 load after its s-tile")
                    mr = sb.tile([128, SN], f32r, name=f"mgr{k}_{j}",
                                 tag="mgr", bufs=2)
                    c_i = nc.vector.tensor_copy(mr[:], st[:])
                    add_dep_helper(c_i.ins, last_insts['dve%d' % min(j + 1, NSN - 1)].ins,
                                   sync=False,
                                   reason="phase4 cast after its s-tile dve")
                    for oc in range(2):
                        nc.tensor.matmul(
                            projp[:, oc * 512:(oc + 1) * 512],
                            wprs[k][:, oc * 128:(oc + 1) * 128],
                            mr[:],
                            start=(k == 0), stop=(k == KCH - 1))
                for oc in range(2):
                    pj = projp[:, oc * 512:(oc + 1) * 512]
                    ost = sb.tile([128, 512], f32, name=f"ost{j}_{oc}",
                                  tag="ost", bufs=3)
                    if has_bias:
                        nc.vector.tensor_scalar_add(ost[:], pj,
                                                    bpr_s[:, oc:oc + 1])
                    else:
                        nc.scalar.copy(out=ost[:], in_=pj)
                    nc.sync.dma_start(
                        out=outT_d[oc * 128:(oc + 1) * 128,
                                   j * SN:(j + 1) * SN],
                        in_=ost[:])

    nc.finalize()
    return nc


def _numpy_fallback(x, mask, past_layer, w_attn, b_attn, w_proj, b_proj):
    qkv = np.einsum("bsd,de->bse", x, w_attn) + b_attn
    q, k, v = np.split(qkv, 3, axis=2)

    def sh(t):
        return t.reshape(B, S, H, DEPTH).transpose(0, 2, 1, 3)

    q, k, v = sh(q), sh(k), sh(v)
    k = np.concatenate([past_layer[:, 0], k], axis=2)
    v = np.concatenate([past_layer[:, 1], v], axis=2)
    present = np.stack([k, v], axis=1)
    scores = np.einsum("bhqd,bhkd->bhqk", q, k) / np.sqrt(np.float32(DEPTH))
    scores = scores + mask * np.float32(-1e9)
    scores = scores - scores.max(axis=-1, keepdims=True)
    e = np.exp(scores)
    attn = e / e.sum(axis=-1, keepdims=True)
    ctx = np.einsum("bhqk,bhkd->bhqd", attn, v)
    merged = ctx.transpose(0, 2, 1, 3).reshape(B, S, D)
    output = np.einsum("bsd,de->bse", merged, w_proj) + b_proj
    return output.astype(np.float32), present.astype(np.float32)


def kernel(x, mask, past_layer, w_attn, b_attn, w_proj, b_proj):
    global LAST_RESULT
    from concourse.bass_utils import run_bass_kernel_spmd

    x = np.asarray(x, dtype=np.float32)
    mask = np.asarray(mask, dtype=np.float32)
    past_layer = np.asarray(past_layer, dtype=np.float32)
    w_attn = np.asarray(w_attn, dtype=np.float32)
    b_attn = np.asarray(b_attn, dtype=np.float32)
    w_proj = np.asarray(w_proj, dtype=np.float32)
    b_proj = np.asarray(b_proj, dtype=np.float32)

    mask2d = np.ascontiguousarray(mask.reshape(S, T))
    mbool = mask2d != 0.0

    # degenerate fully-masked query rows diverge (reference softmax becomes
    # uniform); handle off-device
    if bool(mbool.all(axis=1).any()):
        return _numpy_fallback(x, mask, past_layer, w_attn, b_attn,
                               w_proj, b_proj)

    tri_expect = np.tril(np.ones((128, 128), dtype=np.float32)).T
    diag = 1.0 - mbool[0:128, P:P + 128].T.astype(np.float32)
    if diag.min() == 0.0 and diag.max() == 1.0:
        tri_expect_c = diag
    else:
        tri_expect_c = tri_expect
    plan, mode = _build_plan(mbool, tri_expect_c)

    has_bias = bool(b_attn.any() or b_proj.any())
    key = _plan_key(plan, mode, has_bias)
    if key not in _prog_cache:
        _prog_cache[key] = _build_program(plan, mode, has_bias)
    nc = _prog_cache[key]

    # ---- host-side sharding prep ---------------------------------------
    xT = [np.ascontiguousarray(x[b].T) for b in range(B)]
    in_maps = []
    for c in range(N_CORES):
        b, g = c // 4, c % 4
        hs = list(range(4 * g, 4 * g + 4))
        qcols = np.concatenate([np.arange(64 * h, 64 * h + 64) for h in hs])
        kcols = qcols + D
        vcols = qcols + 2 * D
        m = {
            "xbT": xT[b],
            "wqk": np.ascontiguousarray(
                w_attn[:, np.concatenate([qcols, kcols])]),
            "wv": np.ascontiguousarray(w_attn[:, vcols]),
            "pastKT": np.ascontiguousarray(
                past_layer[b, 0, hs].transpose(0, 2, 1)),
            "pastV": np.ascontiguousarray(past_layer[b, 1, hs]),
            "tri": tri_expect_c,
            "wproj": np.ascontiguousarray(w_proj[:, OC * g:OC * (g + 1)]),
        }
        if has_bias:
            m["bqk"] = np.ascontiguousarray(
                b_attn[np.concatenate([qcols, kcols])])
            m["bv"] = np.ascontiguousarray(b_attn[vcols])
            m["bproj"] = np.ascontiguousarray(b_proj[OC * g:OC * (g + 1)])
        if mode == "general":
            m["multT"] = np.ascontiguousarray(
                (1.0 - mask2d).T.astype(np.float32))
        in_maps.append(m)

    res = run_bass_kernel_spmd(nc, in_maps, list(range(N_CORES)))
    LAST_RESULT = res

    # ---- unshard -------------------------------------------------------
    output = np.empty((B, S, D), dtype=np.float32)
    present = np.empty((B, 2, H, T, DEPTH), dtype=np.float32)
    present[:, 0, :, :P] = past_layer[:, 0]
    present[:, 1, :, :P] = past_layer[:, 1]
    for c in range(N_CORES):
        b, g = c // 4, c % 4
        r = res.results[c]
        output[b, :, OC * g:OC * (g + 1)] = r["outT"].T
        for i, h in enumerate(range(4 * g, 4 * g + 4)):
            present[b, 0, h, P:] = r["pkT"][i].T
            present[b, 1, h, P:] = r["pv"][i]
    return output, present
